# revision 16
# baseline (speedup 1.0000x reference)
"""Trainium2 Bass kernel for the Diversity8 loss.

loss = SCALE * mean_b d[b],   d[b] = (||sum_m v_m[b]||^2 - M) / 2
where v_m[b] = unit-normalized, mean-centered softmax(logits_m[b]/T).

Softmax centering + normalization are shift/scale invariant, so
v_m = (e - mean e) / ||e - mean e||  with  e = exp(x/T) (any overall
scale of e drops out -- we use e' = exp(x/T)/C so the activation's
accum_out IS the mean).

Per (model m, 128-sample group), all stats per partition row:
  - ACT:  e' = Exp(x/T + ln(1/C)), accum_out -> ebar (the mean)
  - var:  m2 = sum (e'-ebar)*e' == sum (e'-ebar)^2  in one fused pass
          (split between ACT Square(e'+(-ebar)) and DVE
          scalar_tensor_tensor for engine balance)
  - DVE:  alpha = rsqrt(m2) via Newton iterations (no ACT table switch)
  - DVE:  ecs = (e' - ebar) * alpha  in ONE 2-scalar tensor_scalar
          (2x perf mode), written as float32r
  - PE :  s += I @ ecs_m accumulated over models in PSUM; identity
          weights make every product exact, so PE adds no multiply noise
  - ACT:  R = sum s^2 via Square with accum_out;  d = 0.5*R - M/2
Centering BEFORE the matmul is load-bearing for accuracy: the loss is a
near-cancelling mean, and any elementwise noise delta on s biases d by
C*var(delta) (chi^2), so all rounding must be relative to |s|~0.1.

Sharding: pure data parallel over the batch dim, 512 samples per core on
8 cores; host sums the per-core [128, 4] d-columns.
"""

import math
import os
import sys

import numpy as np

for _p in ("/opt/trn_rl_repo", "/root/.axon_site/_ro/trn_rl_repo"):
    if os.path.isdir(_p) and _p not in sys.path:
        sys.path.append(_p)

import concourse.bacc as bacc
import concourse.mybir as mybir
from concourse import bass_utils
from concourse.tile import TileContext

F32 = mybir.dt.float32
F32R = mybir.dt.float32r
AF = mybir.ActivationFunctionType
OP = mybir.AluOpType

B = 4096
C = 1000
M = 8
T = 20.0
SCALE = 0.3
N_CORES = 8
B_SHARD = B // N_CORES          # 512 samples per core
G = B_SHARD // 128              # 4 groups of 128 samples
NEWTON_ITERS = 4
# alpha = 1/sqrt(m2), m2 ~ 2.5/C^2 concentrated within ~+-20%
RSQRT_SEED = 0.6324555
# models whose variance pass runs on ACT (rest on DVE) -- engine balance
ACT_VAR_MODELS = (0, 1, 2)
PSUM_SPLITS = ((0, 512), (512, 1000))
MM_DT = F32R                    # matmul operand dtype (full-speed fp32)

_cached = {}


def _build():
    nc = bacc.Bacc("TRN2", target_bir_lowering=False, debug=False)
    xs = [
        nc.dram_tensor(f"x{m}", [B_SHARD, C], F32, kind="ExternalInput")
        for m in range(M)
    ]
    d_dram = nc.dram_tensor("d", [128, G], F32, kind="ExternalOutput")

    with TileContext(nc) as tc:
        with (
            tc.tile_pool(name="const", bufs=1) as const_pool,
            tc.tile_pool(name="x", bufs=4) as x_pool,
            tc.tile_pool(name="e", bufs=12) as e_pool,
            tc.tile_pool(name="ec", bufs=12) as ec_pool,
            tc.tile_pool(name="w", bufs=3) as w_pool,
            tc.tile_pool(name="scr", bufs=3) as scr_pool,
            tc.tile_pool(name="stat", bufs=2) as stat_pool,
            tc.tile_pool(name="dout", bufs=1) as dout_pool,
            tc.tile_pool(name="psum", bufs=2, space="PSUM") as psum_pool,
        ):
            # identity matrix in float32r: I[p, j] = (j == p)
            row_iota = const_pool.tile([128, 128], F32)
            nc.gpsimd.iota(row_iota[:, :], pattern=[[1, 128]], base=0,
                           channel_multiplier=0,
                           allow_small_or_imprecise_dtypes=True)
            p_iota = const_pool.tile([128, 1], F32)
            nc.gpsimd.iota(p_iota[:, :], pattern=[[0, 1]], base=0,
                           channel_multiplier=1,
                           allow_small_or_imprecise_dtypes=True)
            ident = const_pool.tile([128, 128], MM_DT)
            nc.vector.tensor_scalar(ident[:, :], row_iota[:, :],
                                    p_iota[:, :], None, OP.is_equal)

            dout = dout_pool.tile([128, G], F32)

            # keep-warm scratch for the PE (HAM throttles an idle PE to
            # half clock; tiny matmuls between the real bursts keep K=8/8)
            warm_ps = psum_pool.tile([128, 2], F32)


            for g in range(G):
                r0 = g * 128
                s_all = stat_pool.tile([128, M], F32)     # sum e per model
                m2 = stat_pool.tile([128, M], F32)        # sum (e-ebar)^2
                negm = stat_pool.tile([128, M], F32)      # -mean e
                e_tiles = []
                for m in range(M):
                    nc.tensor.matmul(warm_ps[:2, :2], ident[:, :2],
                                     ident[:, :2], start=True, stop=True,
                                     skip_group_check=True)
                    x_t = x_pool.tile([128, C], F32)
                    nc.sync.dma_start(out=x_t[:, :],
                                      in_=xs[m].ap()[r0:r0 + 128, :])
                    e_t = e_pool.tile([128, C], F32)
                    nc.scalar.activation(e_t[:, :], x_t[:, :], AF.Exp,
                                         scale=1.0 / T,
                                         accum_out=s_all[:, m:m + 1])
                    e_tiles.append(e_t)
                # one batched negm per group
                nc.vector.tensor_scalar(negm[:, :], s_all[:, :],
                                        -1.0 / C, None, OP.mult)
                ec_tiles = []
                for m in range(M):
                    # centered ec = e - mean(e); the variance MUST be the
                    # sum of squares of the centered values (quadratically
                    # insensitive to the accumulated-S rounding bias; the
                    # fused (e-eb)*e form leaks that bias into alpha)
                    ec_t = ec_pool.tile([128, C], MM_DT)
                    nc.vector.tensor_scalar(ec_t[:, :], e_tiles[m][:, :],
                                            negm[:, m:m + 1], None, OP.add)
                    scr = scr_pool.tile([128, C], F32)
                    if m in ACT_VAR_MODELS:
                        nc.scalar.activation(scr[:, :], ec_t[:, :], AF.Square,
                                             accum_out=m2[:, m:m + 1])
                    else:
                        nc.vector.scalar_tensor_tensor(
                            scr[:, :], ec_t[:, :], 0.0, ec_t[:, :],
                            op0=OP.bypass, op1=OP.mult,
                            accum_out=m2[:, m:m + 1])
                    ec_tiles.append(ec_t)

                # alpha = rsqrt(m2) by Newton: y <- y * (1.5 - 0.5 * m2 * y^2)
                alpha = stat_pool.tile([128, M], F32)
                tn = stat_pool.tile([128, M], F32)
                nc.vector.memset(alpha[:, :], RSQRT_SEED)
                for _ in range(NEWTON_ITERS):
                    nc.vector.scalar_tensor_tensor(
                        tn[:, :], alpha[:, :], 0.0, alpha[:, :],
                        op0=OP.bypass, op1=OP.mult)            # y^2
                    nc.vector.scalar_tensor_tensor(
                        tn[:, :], tn[:, :], -0.5, m2[:, :],
                        op0=OP.mult, op1=OP.mult)              # -0.5*y^2*m2
                    nc.vector.scalar_tensor_tensor(
                        alpha[:, :], tn[:, :], 1.5, alpha[:, :],
                        op0=OP.add, op1=OP.mult)               # y*(1.5+that)
                # s = sum_m diag(alpha_m) @ ec_m  (PSUM accumulation)
                s_ps = psum_pool.tile([128, C], F32)
                for m in range(M):
                    w_t = w_pool.tile([128, 128], MM_DT)
                    nc.vector.tensor_scalar(w_t[:, :], ident[:, :],
                                            alpha[:, m:m + 1], None, OP.mult)
                    for c0, c1 in PSUM_SPLITS:
                        nc.tensor.matmul(s_ps[:, c0:c1],
                                         w_t[:, :],
                                         ec_tiles[m][:, c0:c1],
                                         start=(m == 0), stop=(m == M - 1))

                # d = 0.5 * sum s^2 - M/2
                r_col = stat_pool.tile([128, 1], F32)
                scr2 = scr_pool.tile([128, C], F32)
                nc.scalar.activation(scr2[:, :], s_ps[:, :], AF.Square,
                                     accum_out=r_col[:, :])
                nc.vector.tensor_scalar(dout[:, g:g + 1], r_col[:, :],
                                        0.5, -M / 2.0, OP.mult, OP.add)

            nc.sync.dma_start(out=d_dram.ap(), in_=dout[:, :])

    nc.compile()
    return nc


def _get_nc():
    if "nc" not in _cached:
        _cached["nc"] = _build()
    return _cached["nc"]


def kernel(**inputs: np.ndarray) -> np.ndarray:
    nc = _get_nc()
    outs = [np.asarray(inputs[f"outputs{m + 1}"], dtype=np.float32)
            for m in range(M)]
    in_maps = []
    for c in range(N_CORES):
        sl = slice(c * B_SHARD, (c + 1) * B_SHARD)
        in_maps.append(
            {f"x{m}": np.ascontiguousarray(outs[m][sl]) for m in range(M)}
        )
    res = bass_utils.run_bass_kernel_spmd(nc, in_maps,
                                          core_ids=list(range(N_CORES)))
    total = 0.0
    for c in range(N_CORES):
        total += float(res.results[c]["d"].astype(np.float64).sum())
    return np.array(SCALE * total / B, dtype=np.float32)


# revision 17
# speedup vs baseline: 1.0418x; 1.0418x over previous
"""Trainium2 Bass kernel for the Diversity8 loss.

loss = SCALE * mean_b d[b],   d[b] = (||sum_m v_m[b]||^2 - M) / 2
where v_m[b] = unit-normalized, mean-centered softmax(logits_m[b]/T).

Softmax centering + normalization are shift/scale invariant, so
v_m = (e - mean e) / ||e - mean e||  with  e = exp(x/T).

Per (model m, 128-sample group), all stats per partition row:
  - ACT:  e = Exp(x/T), accum_out -> S = sum e
  - DVE:  ec = e - S/C  (centered tile, written as float32r)
  - var:  m2 = sum ec^2, split between ACT Square and DVE
    scalar_tensor_tensor for engine balance.  The SQUARED form is
    load-bearing: sum (e-eb)*e leaks the accumulated-S rounding bias
    linearly into alpha, which biases the near-cancelling loss mean.
  - DVE:  alpha = rsqrt(m2) via Newton (batched over group PAIRS; no
    ACT table switch, and one-sided convergence means iterations must
    run to fp32 convergence or alpha is biased low)
  - PE :  s += diag(alpha_m) @ ec_m accumulated over models in PSUM
    (float32r single-pass matmuls; centering BEFORE the matmul keeps
    the PE's reduced-precision product noise relative to |s|~0.1 --
    elementwise noise delta on s biases d by C*var(delta), chi^2)
  - ACT:  R = sum s^2 via Square with accum_out;  d = 0.5*R - M/2

Sharding: pure data parallel over the batch dim, 512 samples per core on
8 cores; host sums the per-core [128, 4] d-columns.
"""

import os
import sys

import numpy as np

for _p in ("/opt/trn_rl_repo", "/root/.axon_site/_ro/trn_rl_repo"):
    if os.path.isdir(_p) and _p not in sys.path:
        sys.path.append(_p)

import concourse.bacc as bacc
import concourse.mybir as mybir
from concourse import bass_utils
from concourse.tile import TileContext

F32 = mybir.dt.float32
F32R = mybir.dt.float32r
AF = mybir.ActivationFunctionType
OP = mybir.AluOpType

B = 4096
C = 1000
M = 8
T = 20.0
SCALE = 0.3
N_CORES = 8
B_SHARD = B // N_CORES          # 512 samples per core
G = B_SHARD // 128              # 4 groups of 128 samples
GP = G // 2                     # group pairs (Newton batched per pair)
NEWTON_ITERS = 4
RSQRT_SEED = 0.6324555          # ~ 1/sqrt(2.5); m2 concentrates in [2.0,3.1]
# models whose variance pass runs on ACT (rest on DVE) -- engine balance
ACT_VAR_MODELS = (0, 1, 2)
PSUM_SPLITS = ((0, 512), (512, 1000))
MM_DT = F32R                    # matmul operand dtype (full-speed fp32)

_cached = {}


def _build():
    nc = bacc.Bacc("TRN2", target_bir_lowering=False, debug=False)
    xs = [
        nc.dram_tensor(f"x{m}", [B_SHARD, C], F32, kind="ExternalInput")
        for m in range(M)
    ]
    ident_dram = nc.dram_tensor("ident", [128, 128], F32,
                                kind="ExternalInput")
    d_dram = nc.dram_tensor("d", [128, G], F32, kind="ExternalOutput")

    with TileContext(nc) as tc:
        with (
            tc.tile_pool(name="const", bufs=1) as const_pool,
            tc.tile_pool(name="x", bufs=5) as x_pool,
            tc.tile_pool(name="e", bufs=11) as e_pool,
            tc.tile_pool(name="ec", bufs=18) as ec_pool,
            tc.tile_pool(name="w", bufs=3) as w_pool,
            tc.tile_pool(name="scr", bufs=3) as scr_pool,
            tc.tile_pool(name="stat", bufs=2) as stat_pool,
            tc.tile_pool(name="dout", bufs=1) as dout_pool,
            tc.tile_pool(name="psum", bufs=2, space="PSUM") as psum_pool,
        ):
            ident = const_pool.tile([128, 128], MM_DT)
            nc.sync.dma_start(out=ident[:, :],
                              in_=ident_dram.ap().bitcast(F32R))

            dout = dout_pool.tile([128, G], F32)
            rall = dout_pool.tile([128, G], F32)

            for gp in range(GP):
                MP = 2 * M
                s_all = stat_pool.tile([128, MP], F32)   # sum e per (g,m)
                negm = stat_pool.tile([128, MP], F32)    # -mean e
                m2 = stat_pool.tile([128, MP], F32)      # sum (e-ebar)^2
                e_tiles = {}
                ec_tiles = {}
                for gl in range(2):
                    g = gp * 2 + gl
                    r0 = g * 128
                    for m in range(M):
                        j = gl * M + m
                        x_t = x_pool.tile([128, C], F32)
                        nc.sync.dma_start(out=x_t[:, :],
                                          in_=xs[m].ap()[r0:r0 + 128, :])
                        e_t = e_pool.tile([128, C], F32)
                        nc.scalar.activation(e_t[:, :], x_t[:, :], AF.Exp,
                                             scale=1.0 / T,
                                             accum_out=s_all[:, j:j + 1])
                        e_tiles[j] = e_t
                    # batched negm for this group's 8 columns
                    c0 = gl * M
                    nc.vector.tensor_scalar(negm[:, c0:c0 + M],
                                            s_all[:, c0:c0 + M],
                                            -1.0 / C, None, OP.mult)
                    for m in range(M):
                        j = gl * M + m
                        ec_t = ec_pool.tile([128, C], MM_DT)
                        nc.vector.tensor_scalar(ec_t[:, :],
                                                e_tiles[j][:, :],
                                                negm[:, j:j + 1], None,
                                                OP.add)
                        scr = scr_pool.tile([128, C], F32)
                        if m in ACT_VAR_MODELS:
                            nc.scalar.activation(scr[:, :], ec_t[:, :],
                                                 AF.Square,
                                                 accum_out=m2[:, j:j + 1])
                        else:
                            nc.vector.scalar_tensor_tensor(
                                scr[:, :], ec_t[:, :], 0.0, ec_t[:, :],
                                op0=OP.bypass, op1=OP.mult,
                                accum_out=m2[:, j:j + 1])
                        ec_tiles[j] = ec_t

                # alpha = rsqrt(m2), Newton batched over both groups:
                # y <- y * (1.5 - 0.5 * m2 * y^2)
                alpha = stat_pool.tile([128, MP], F32)
                tn = stat_pool.tile([128, MP], F32)
                nc.vector.memset(alpha[:, :], RSQRT_SEED)
                for _ in range(NEWTON_ITERS):
                    nc.vector.scalar_tensor_tensor(
                        tn[:, :], alpha[:, :], 0.0, alpha[:, :],
                        op0=OP.bypass, op1=OP.mult)            # y^2
                    nc.vector.scalar_tensor_tensor(
                        tn[:, :], tn[:, :], -0.5, m2[:, :],
                        op0=OP.mult, op1=OP.mult)              # -0.5*y^2*m2
                    nc.vector.scalar_tensor_tensor(
                        alpha[:, :], tn[:, :], 1.5, alpha[:, :],
                        op0=OP.add, op1=OP.mult)               # y*(1.5+that)

                for gl in range(2):
                    g = gp * 2 + gl
                    # s = sum_m diag(alpha_m) @ ec_m  (PSUM accumulation)
                    s_ps = psum_pool.tile([128, C], F32)
                    for m in range(M):
                        j = gl * M + m
                        w_t = w_pool.tile([128, 128], MM_DT)
                        nc.vector.tensor_scalar(w_t[:, :], ident[:, :],
                                                alpha[:, j:j + 1], None,
                                                OP.mult)
                        for c0, c1 in PSUM_SPLITS:
                            nc.tensor.matmul(s_ps[:, c0:c1],
                                             w_t[:, :],
                                             ec_tiles[j][:, c0:c1],
                                             start=(m == 0),
                                             stop=(m == M - 1))

                    # R = sum s^2
                    scr2 = scr_pool.tile([128, C], F32)
                    nc.scalar.activation(scr2[:, :], s_ps[:, :], AF.Square,
                                         accum_out=rall[:, g:g + 1])

            # d = 0.5 * R - M/2 for all groups at once
            nc.vector.tensor_scalar(dout[:, :], rall[:, :],
                                    0.5, -M / 2.0, OP.mult, OP.add)
            nc.sync.dma_start(out=d_dram.ap(), in_=dout[:, :])

    nc.compile()
    return nc


def _get_nc():
    if "nc" not in _cached:
        _cached["nc"] = _build()
    return _cached["nc"]


_IDENT = np.eye(128, dtype=np.float32)


def kernel(**inputs: np.ndarray) -> np.ndarray:
    nc = _get_nc()
    outs = [np.asarray(inputs[f"outputs{m + 1}"], dtype=np.float32)
            for m in range(M)]
    in_maps = []
    for c in range(N_CORES):
        sl = slice(c * B_SHARD, (c + 1) * B_SHARD)
        im = {f"x{m}": np.ascontiguousarray(outs[m][sl]) for m in range(M)}
        im["ident"] = _IDENT
        in_maps.append(im)
    res = bass_utils.run_bass_kernel_spmd(nc, in_maps,
                                          core_ids=list(range(N_CORES)))
    total = 0.0
    for c in range(N_CORES):
        total += float(res.results[c]["d"].astype(np.float64).sum())
    return np.array(SCALE * total / B, dtype=np.float32)


# revision 18
# speedup vs baseline: 1.1390x; 1.0934x over previous
"""Trainium2 Bass kernel for the Diversity8 loss.

loss = SCALE * mean_b d[b],   d[b] = (||sum_m v_m[b]||^2 - M) / 2
where v_m[b] = unit-normalized, mean-centered softmax(logits_m[b]/T).

Softmax centering + normalization are shift/scale invariant, so
v_m = (e - mean e) / ||e - mean e||  with  e = exp(x/T).

Per (model m, 128-sample group), all stats per partition row:
  - ACT:  e = Exp(x/T), accum_out -> S = sum e
  - DVE:  ec = e - S/C  (centered tile, written as float32r)
  - var:  m2 = sum ec^2, split between ACT Square and DVE
    scalar_tensor_tensor for engine balance.  The SQUARED form is
    load-bearing: sum (e-eb)*e leaks the accumulated-S rounding bias
    linearly into alpha, which biases the near-cancelling loss mean.
  - DVE:  alpha = rsqrt(m2) via Newton from a linear seed (no ACT
    table switch; one-sided convergence means iterations must run to
    fp32 convergence or alpha is biased low)
  - PE :  s += diag(alpha_m) @ ec_m accumulated over models in PSUM
    (float32r single-pass matmuls; centering BEFORE the matmul keeps
    the PE's reduced-precision product noise relative to |s|~0.1 --
    elementwise noise delta on s biases d by C*var(delta), chi^2)
  - ACT:  R = sum s^2 via Square with accum_out;  d = 0.5*R - M/2

Sharding: pure data parallel over the batch dim, 512 samples per core on
8 cores; host sums the per-core [128, 4] d-columns.
"""

import os
import sys

import numpy as np

for _p in ("/opt/trn_rl_repo", "/root/.axon_site/_ro/trn_rl_repo"):
    if os.path.isdir(_p) and _p not in sys.path:
        sys.path.append(_p)

import concourse.bacc as bacc
import concourse.mybir as mybir
from concourse import bass_utils
from concourse.tile import TileContext

F32 = mybir.dt.float32
F32R = mybir.dt.float32r
AF = mybir.ActivationFunctionType
OP = mybir.AluOpType

B = 4096
C = 1000
M = 8
T = 20.0
SCALE = 0.3
N_CORES = 8
B_SHARD = B // N_CORES          # 512 samples per core
G = B_SHARD // 128              # 4 groups of 128 samples
NEWTON_ITERS = 3
# linear seed y0 = SEED_A + SEED_B*m2 fitted at m2=2.5 (m2 concentrates
# in [2.0, 3.1]); |eps0| <= ~2% so 3 Newton iters converge to fp32 exact
SEED_A = 0.94868
SEED_B = -0.126491
# per-group ACT-variance model count (rest on DVE): later groups lean on
# ACT because it has run out of exp work while DVE drains the tail
ACT_VAR_COUNT = (1, 2, 2, 4)
PSUM_SPLITS = ((0, 512), (512, 1000))
MM_DT = F32R                    # matmul operand dtype (full-speed fp32)

_cached = {}


def _build():
    nc = bacc.Bacc("TRN2", target_bir_lowering=False, debug=False)
    xs = [
        nc.dram_tensor(f"x{m}", [B_SHARD, C], F32, kind="ExternalInput")
        for m in range(M)
    ]
    ident_dram = nc.dram_tensor("ident", [128, 128], F32,
                                kind="ExternalInput")
    d_dram = nc.dram_tensor("d", [128, G], F32, kind="ExternalOutput")

    with TileContext(nc) as tc:
        with (
            tc.tile_pool(name="const", bufs=1) as const_pool,
            tc.tile_pool(name="x", bufs=5) as x_pool,
            tc.tile_pool(name="e", bufs=11) as e_pool,
            tc.tile_pool(name="ec", bufs=12) as ec_pool,
            tc.tile_pool(name="w", bufs=3) as w_pool,
            tc.tile_pool(name="scr", bufs=3) as scr_pool,
            tc.tile_pool(name="stat", bufs=2) as stat_pool,
            tc.tile_pool(name="dout", bufs=1) as dout_pool,
            tc.tile_pool(name="psum", bufs=2, space="PSUM") as psum_pool,
        ):
            ident = const_pool.tile([128, 128], MM_DT)
            nc.sync.dma_start(out=ident[:, :],
                              in_=ident_dram.ap().bitcast(F32R))

            dout = dout_pool.tile([128, G], F32)
            rall = dout_pool.tile([128, G], F32)

            H = M // 2           # alpha computed per half-group of models
            for g in range(G):
                r0 = g * 128
                s_all = stat_pool.tile([128, M], F32)    # sum e per model
                negm = stat_pool.tile([128, M], F32)     # -mean e
                m2 = stat_pool.tile([128, M], F32)       # sum (e-ebar)^2
                alpha = stat_pool.tile([128, M], F32)
                tn = stat_pool.tile([128, M], F32)
                e_tiles = {}
                ec_tiles = {}
                n_act_var = ACT_VAR_COUNT[g]
                s_ps = psum_pool.tile([128, C], F32)
                for h in range(2):
                    h0 = h * H
                    for m in range(h0, h0 + H):
                        x_t = x_pool.tile([128, C], F32)
                        nc.sync.dma_start(out=x_t[:, :],
                                          in_=xs[m].ap()[r0:r0 + 128, :])
                        e_t = e_pool.tile([128, C], F32)
                        nc.scalar.activation(e_t[:, :], x_t[:, :], AF.Exp,
                                             scale=1.0 / T,
                                             accum_out=s_all[:, m:m + 1])
                        e_tiles[m] = e_t
                    nc.vector.tensor_scalar(negm[:, h0:h0 + H],
                                            s_all[:, h0:h0 + H],
                                            -1.0 / C, None, OP.mult)
                    for m in range(h0, h0 + H):
                        ec_t = ec_pool.tile([128, C], MM_DT)
                        nc.vector.tensor_scalar(ec_t[:, :],
                                                e_tiles[m][:, :],
                                                negm[:, m:m + 1], None,
                                                OP.add)
                        scr = scr_pool.tile([128, C], F32)
                        if (m % H) < (n_act_var + 1) // 2:
                            nc.scalar.activation(scr[:, :], ec_t[:, :],
                                                 AF.Square,
                                                 accum_out=m2[:, m:m + 1])
                        else:
                            nc.vector.scalar_tensor_tensor(
                                scr[:, :], ec_t[:, :], 0.0, ec_t[:, :],
                                op0=OP.bypass, op1=OP.mult,
                                accum_out=m2[:, m:m + 1])
                        ec_tiles[m] = ec_t

                    # alpha = rsqrt(m2) for this half: linear seed then
                    # Newton y <- y * (1.5 - 0.5 * m2 * y^2)
                    hs = slice(h0, h0 + H)
                    nc.vector.tensor_scalar(alpha[:, hs], m2[:, hs],
                                            SEED_B, SEED_A,
                                            OP.mult, OP.add)
                    for _ in range(NEWTON_ITERS):
                        nc.vector.scalar_tensor_tensor(
                            tn[:, hs], alpha[:, hs], 0.0, alpha[:, hs],
                            op0=OP.bypass, op1=OP.mult)        # y^2
                        nc.vector.scalar_tensor_tensor(
                            tn[:, hs], tn[:, hs], -0.5, m2[:, hs],
                            op0=OP.mult, op1=OP.mult)          # -.5*y^2*m2
                        nc.vector.scalar_tensor_tensor(
                            alpha[:, hs], tn[:, hs], 1.5, alpha[:, hs],
                            op0=OP.add, op1=OP.mult)           # y*(1.5+t)

                    # s += sum_m diag(alpha_m) @ ec_m  (PSUM accumulation)
                    for m in range(h0, h0 + H):
                        w_t = w_pool.tile([128, 128], MM_DT)
                        nc.vector.tensor_scalar(w_t[:, :], ident[:, :],
                                                alpha[:, m:m + 1], None,
                                                OP.mult)
                        for c0, c1 in PSUM_SPLITS:
                            nc.tensor.matmul(s_ps[:, c0:c1],
                                             w_t[:, :],
                                             ec_tiles[m][:, c0:c1],
                                             start=(m == 0),
                                             stop=(m == M - 1))

                # R = sum s^2
                scr2 = scr_pool.tile([128, C], F32)
                nc.scalar.activation(scr2[:, :], s_ps[:, :], AF.Square,
                                     accum_out=rall[:, g:g + 1])

            # d = 0.5 * R - M/2 for all groups at once
            nc.vector.tensor_scalar(dout[:, :], rall[:, :],
                                    0.5, -M / 2.0, OP.mult, OP.add)
            nc.sync.dma_start(out=d_dram.ap(), in_=dout[:, :])

    nc.compile()
    return nc


def _get_nc():
    if "nc" not in _cached:
        _cached["nc"] = _build()
    return _cached["nc"]


_IDENT = np.eye(128, dtype=np.float32)


def kernel(**inputs: np.ndarray) -> np.ndarray:
    nc = _get_nc()
    outs = [np.asarray(inputs[f"outputs{m + 1}"], dtype=np.float32)
            for m in range(M)]
    in_maps = []
    for c in range(N_CORES):
        sl = slice(c * B_SHARD, (c + 1) * B_SHARD)
        im = {f"x{m}": np.ascontiguousarray(outs[m][sl]) for m in range(M)}
        im["ident"] = _IDENT
        in_maps.append(im)
    res = bass_utils.run_bass_kernel_spmd(nc, in_maps,
                                          core_ids=list(range(N_CORES)))
    total = 0.0
    for c in range(N_CORES):
        total += float(res.results[c]["d"].astype(np.float64).sum())
    return np.array(SCALE * total / B, dtype=np.float32)


# revision 21
# speedup vs baseline: 1.1695x; 1.0268x over previous
"""Trainium2 Bass kernel for the Diversity8 loss.

loss = SCALE * mean_b d[b],   d[b] = (||sum_m v_m[b]||^2 - M) / 2
where v_m[b] = unit-normalized, mean-centered softmax(logits_m[b]/T).

Softmax centering + normalization are shift/scale invariant, so
v_m = (e - mean e) / ||e - mean e||  with  e = exp(x/T).

Per (model m, 128-sample group), all stats per partition row:
  - ACT:  e = Exp(x/T), accum_out -> S = sum e
  - DVE:  ec = e - S/C  (centered tile, written as float32r)
  - var:  m2 = sum ec^2, split between ACT Square and DVE
    scalar_tensor_tensor for engine balance.  The SQUARED form is
    load-bearing: sum (e-eb)*e leaks the accumulated-S rounding bias
    linearly into alpha, which biases the near-cancelling loss mean.
  - DVE:  alpha = rsqrt(m2) via Newton from a linear seed (no ACT
    table switch; one-sided convergence means iterations must run to
    fp32 convergence or alpha is biased low)
  - PE :  s += diag(alpha_m) @ ec_m accumulated over models in PSUM
    (float32r single-pass matmuls; centering BEFORE the matmul keeps
    the PE's reduced-precision product noise relative to |s|~0.1 --
    elementwise noise delta on s biases d by C*var(delta), chi^2)
  - ACT:  R = sum s^2 via Square with accum_out;  d = 0.5*R - M/2

Sharding: pure data parallel over the batch dim, 512 samples per core on
8 cores; host sums the per-core [128, 4] d-columns.
"""

import os
import sys

import numpy as np

for _p in ("/opt/trn_rl_repo", "/root/.axon_site/_ro/trn_rl_repo"):
    if os.path.isdir(_p) and _p not in sys.path:
        sys.path.append(_p)

import concourse.bacc as bacc
import concourse.mybir as mybir
from concourse import bass_utils
from concourse.tile import TileContext

F32 = mybir.dt.float32
F32R = mybir.dt.float32r
AF = mybir.ActivationFunctionType
OP = mybir.AluOpType

B = 4096
C = 1000
M = 8
T = 20.0
SCALE = 0.3
N_CORES = 8
B_SHARD = B // N_CORES          # 512 samples per core
G = B_SHARD // 128              # 4 groups of 128 samples
NEWTON_ITERS = 3
# linear seed y0 = SEED_A + SEED_B*m2 fitted at m2=2.5 (m2 concentrates
# in [2.0, 3.1]); |eps0| <= ~2% so 3 Newton iters converge to fp32 exact
SEED_A = 0.94868
SEED_B = -0.126491
# per-group ACT-variance model count (rest on DVE): later groups lean on
# ACT because it has run out of exp work while DVE drains the tail
ACT_VAR_COUNT = (1, 2, 2, 4)
PSUM_SPLITS = ((0, 512), (512, 1000))
MM_DT = F32R                    # matmul operand dtype (full-speed fp32)

_cached = {}


def _build():
    nc = bacc.Bacc("TRN2", target_bir_lowering=False, debug=False)
    xs = [
        nc.dram_tensor(f"x{m}", [B_SHARD, C], F32, kind="ExternalInput")
        for m in range(M)
    ]
    ident_dram = nc.dram_tensor("ident", [128, 128], F32,
                                kind="ExternalInput")
    d_dram = nc.dram_tensor("d", [128, G], F32, kind="ExternalOutput")

    with TileContext(nc) as tc:
        with (
            tc.tile_pool(name="const", bufs=1) as const_pool,
            tc.tile_pool(name="x", bufs=5) as x_pool,
            tc.tile_pool(name="e", bufs=11) as e_pool,
            tc.tile_pool(name="ec", bufs=12) as ec_pool,
            tc.tile_pool(name="w", bufs=3) as w_pool,
            tc.tile_pool(name="scr", bufs=3) as scr_pool,
            tc.tile_pool(name="stat", bufs=2) as stat_pool,
            tc.tile_pool(name="dout", bufs=1) as dout_pool,
            tc.tile_pool(name="psum", bufs=3, space="PSUM") as psum_pool,
            tc.tile_pool(name="wps", bufs=2, space="PSUM") as warm_pool,
        ):
            ident = const_pool.tile([128, 128], MM_DT)
            nc.sync.dma_start(out=ident[:, :],
                              in_=ident_dram.ap().bitcast(F32R))

            dout = dout_pool.tile([128, G], F32)
            rall = dout_pool.tile([128, G], F32)

            H = M // 2           # alpha computed per half-group of models
            for g in range(G):
                r0 = g * 128
                s_all = stat_pool.tile([128, M], F32)    # sum e per model
                negm = stat_pool.tile([128, M], F32)     # -mean e
                m2 = stat_pool.tile([128, M], F32)       # sum (e-ebar)^2
                alpha = stat_pool.tile([128, M], F32)
                tn = stat_pool.tile([128, M], F32)
                e_tiles = {}
                ec_tiles = {}
                n_act_var = ACT_VAR_COUNT[g]
                s_ps = psum_pool.tile([128, C], F32)
                for h in range(2):
                    h0 = h * H
                    for m in range(h0, h0 + H):
                        warm_t = warm_pool.tile([2, 2], F32)
                        nc.tensor.matmul(warm_t[:, :], ident[:, :2],
                                         ident[:, :2], start=True, stop=True,
                                         skip_group_check=True)
                        x_t = x_pool.tile([128, C], F32)
                        nc.sync.dma_start(out=x_t[:, :],
                                          in_=xs[m].ap()[r0:r0 + 128, :])
                        e_t = e_pool.tile([128, C], F32)
                        nc.scalar.activation(e_t[:, :], x_t[:, :], AF.Exp,
                                             scale=1.0 / T,
                                             accum_out=s_all[:, m:m + 1])
                        e_tiles[m] = e_t
                    nc.vector.tensor_scalar(negm[:, h0:h0 + H],
                                            s_all[:, h0:h0 + H],
                                            -1.0 / C, None, OP.mult)
                    for m in range(h0, h0 + H):
                        ec_t = ec_pool.tile([128, C], MM_DT)
                        nc.vector.tensor_scalar(ec_t[:, :],
                                                e_tiles[m][:, :],
                                                negm[:, m:m + 1], None,
                                                OP.add)
                        scr = scr_pool.tile([128, C], F32)
                        if (m % H) < (n_act_var + 1) // 2:
                            nc.scalar.activation(scr[:, :], ec_t[:, :],
                                                 AF.Square,
                                                 accum_out=m2[:, m:m + 1])
                        else:
                            nc.vector.scalar_tensor_tensor(
                                scr[:, :], ec_t[:, :], 0.0, ec_t[:, :],
                                op0=OP.bypass, op1=OP.mult,
                                accum_out=m2[:, m:m + 1])
                        ec_tiles[m] = ec_t

                    # alpha = rsqrt(m2) for this half: linear seed then
                    # Newton y <- y * (1.5 - 0.5 * m2 * y^2)
                    hs = slice(h0, h0 + H)
                    nc.vector.tensor_scalar(alpha[:, hs], m2[:, hs],
                                            SEED_B, SEED_A,
                                            OP.mult, OP.add)
                    for _ in range(NEWTON_ITERS):
                        nc.vector.scalar_tensor_tensor(
                            tn[:, hs], alpha[:, hs], 0.0, alpha[:, hs],
                            op0=OP.bypass, op1=OP.mult)        # y^2
                        nc.vector.scalar_tensor_tensor(
                            tn[:, hs], tn[:, hs], -0.5, m2[:, hs],
                            op0=OP.mult, op1=OP.mult)          # -.5*y^2*m2
                        nc.vector.scalar_tensor_tensor(
                            alpha[:, hs], tn[:, hs], 1.5, alpha[:, hs],
                            op0=OP.add, op1=OP.mult)           # y*(1.5+t)

                    # s += sum_m diag(alpha_m) @ ec_m  (PSUM accumulation)
                    for m in range(h0, h0 + H):
                        w_t = w_pool.tile([128, 128], MM_DT)
                        nc.vector.tensor_scalar(w_t[:, :], ident[:, :],
                                                alpha[:, m:m + 1], None,
                                                OP.mult)
                        for c0, c1 in PSUM_SPLITS:
                            nc.tensor.matmul(s_ps[:, c0:c1],
                                             w_t[:, :],
                                             ec_tiles[m][:, c0:c1],
                                             start=(m == 0),
                                             stop=(m == M - 1))

                # R = sum s^2
                scr2 = scr_pool.tile([128, C], F32)
                nc.scalar.activation(scr2[:, :], s_ps[:, :], AF.Square,
                                     accum_out=rall[:, g:g + 1])

            # d = 0.5 * R - M/2 for all groups at once
            nc.vector.tensor_scalar(dout[:, :], rall[:, :],
                                    0.5, -M / 2.0, OP.mult, OP.add)
            nc.sync.dma_start(out=d_dram.ap(), in_=dout[:, :])

    nc.compile()
    return nc


def _get_nc():
    if "nc" not in _cached:
        _cached["nc"] = _build()
    return _cached["nc"]


_IDENT = np.eye(128, dtype=np.float32)


def kernel(**inputs: np.ndarray) -> np.ndarray:
    nc = _get_nc()
    outs = [np.asarray(inputs[f"outputs{m + 1}"], dtype=np.float32)
            for m in range(M)]
    in_maps = []
    for c in range(N_CORES):
        sl = slice(c * B_SHARD, (c + 1) * B_SHARD)
        im = {f"x{m}": np.ascontiguousarray(outs[m][sl]) for m in range(M)}
        im["ident"] = _IDENT
        in_maps.append(im)
    res = bass_utils.run_bass_kernel_spmd(nc, in_maps,
                                          core_ids=list(range(N_CORES)))
    total = 0.0
    for c in range(N_CORES):
        total += float(res.results[c]["d"].astype(np.float64).sum())
    return np.array(SCALE * total / B, dtype=np.float32)
